# revision 6
# baseline (speedup 1.0000x reference)
"""Trainium2 Bass kernel: per-sample hypernetwork depthwise 3x3 conv.

Reference computation (per batch b):
    W_dw[b] = (z[b] @ W_lin.T).reshape(OUT_C, 1, 3, 3)
    y[b]    = depthwise_conv2d(x[b], W_dw[b], padding=1)

Sharding: data-parallel over batch across 8 NeuronCores (2 batches/core),
W_lin replicated. Each core computes its own W_dw on-device.

Per-core v2 design (fp16 on-chip, fp32 in HBM):
  - channels (256) -> 2 groups of 128 on SBUF partitions
  - image split into 64-row bands; each band (66 rows incl 1-row halo each
    side) is DMA'd HBM->SBUF with an inline fp32->fp16 cast (SWDGE/gpsimd),
    into a flat tile (rows back to back, 2 leading pad elems). Width-edge
    taps wrap into the neighboring row; those wrong contributions are
    subtracted afterwards by small correction ops with negated weights.
  - per 16-row output group (2048 elems), the 9 conv taps split:
      * 6 taps on PE: diagonal-weight fp16 matmuls accumulating in PSUM
      * 1 tap on DVE tensor_scalar (4x perf mode) - first write of og
      * 2 taps on ACT (scalar engine) multiplies into scratch, added to og
        by DVE tensor_tensor (2x perf mode)
      * PSUM drained by one DVE tensor_tensor add (1x, psum operand)
  - output band DMA'd SBUF->HBM with inline fp16->fp32 cast (SWDGE)
  - W_dw computed on-device by 18 small fp32 matmuls from a host-side
    re-layout of W_lin (pure permutation/transpose, no host math)
"""

import os
import sys

for _p in ("/opt/trn_rl_repo", "/root/.axon_site", "/root/.axon_site/_ro/trn_rl_repo",
           "/root/.axon_site/_ro/pypackages"):
    if os.path.isdir(_p) and _p not in sys.path:
        sys.path.append(_p)

import numpy as np

import concourse.bass as bass
import concourse.tile as tile
from concourse import bacc, mybir
from concourse import bass_utils
from concourse.alu_op_type import AluOpType

F32 = mybir.dt.float32
F16 = mybir.dt.float16

# problem constants (hardcoded per contract)
B, OUT_C, H, W = 16, 256, 128, 128
K, Z_DIM = 3, 64
N_CORES = 8
B_PER = B // N_CORES          # 2 batches per core
G = OUT_C // 128              # 2 channel groups of 128

ROWS_BAND = 64
ROWS_GROUP = 16
PAD = 2                       # leading pad elems in the flat band tile

TAPS = [(dy, dx) for dy in range(3) for dx in range(3)]
PE_TAPS = [0, 2, 3, 5, 6, 8]  # dx in {0,2}
TS_TAP = 1                    # (dy=0, dx=1) -> DVE tensor_scalar first write
ACT_TAPS = [4, 7]             # (dy,dx)=(1,1),(2,1) -> ACT mul + DVE add


def build_nc(rows_band=ROWS_BAND, rows_group=ROWS_GROUP, b_per=B_PER, h=H):
    n_bands = h // rows_band
    grp_per_band = rows_band // rows_group
    banks_per_grp = rows_group * W // 512   # 4
    grp_free = rows_group * W               # 2048
    band_free = rows_band * W               # 8192
    tile_rows = rows_band + 2               # 66
    flat_n = PAD + tile_rows * W + 2 + W    # slack for correction APs

    nc = bacc.Bacc("TRN2", target_bir_lowering=False, debug=False)

    x_d = nc.dram_tensor("x", [b_per, OUT_C, h, W], F32, kind="ExternalInput")
    zt_d = nc.dram_tensor("zT", [Z_DIM, b_per], F32, kind="ExternalInput")
    wlt_d = nc.dram_tensor("wlt", [Z_DIM, OUT_C * K * K], F32, kind="ExternalInput")
    ident_d = nc.dram_tensor("ident", [128, 128], F32, kind="ExternalInput")
    y_d = nc.dram_tensor("y", [b_per, OUT_C, h, W], F32, kind="ExternalOutput")

    n_chunks = OUT_C * K * K // 128          # 18
    wd_cols = K * K * G * b_per              # 36, col = (g*9 + t)*b_per + b

    with tile.TileContext(nc) as tc:
        with tc.tile_pool(name="wconst", bufs=1) as wpool:
            ident = wpool.tile([128, 128], F32)
            nc.sync.dma_start(ident[:], ident_d.ap()[:, :])
            wlt = wpool.tile([Z_DIM, OUT_C * K * K], F32)
            half = OUT_C * K * K // 2
            nc.sync.dma_start(wlt[:, 0:half], wlt_d.ap()[:, 0:half])
            nc.sync.dma_start(wlt[:, half:], wlt_d.ap()[:, half:])
            zt = wpool.tile([Z_DIM, b_per], F32)
            nc.sync.dma_start(zt[:], zt_d.ap()[:, :])

            wd = wpool.tile([128, wd_cols], F32)
            with tc.tile_pool(name="wpsum", bufs=2, space="PSUM") as wps:
                for j in range(n_chunks):
                    ps = wps.tile([128, b_per], F32)
                    nc.tensor.matmul(ps[:], wlt[:, 128 * j:128 * (j + 1)], zt[:],
                                     start=True, stop=True)
                    nc.scalar.copy(wd[:, b_per * j:b_per * (j + 1)], ps[:])

            # negated weights for the wrap corrections
            wdn = wpool.tile([128, wd_cols], F32)
            nc.scalar.mul(wdn[:], wd[:], -1.0)

            # fp16 diagonal weight matrices for the PE taps (built lazily
            # per (b,g) right before that image's bands, see loop below)
            diags = {}

            with tc.tile_pool(name="xband", bufs=6) as xpool, \
                 tc.tile_pool(name="oband", bufs=3) as opool, \
                 tc.tile_pool(name="scr", bufs=5) as scpool, \
                 tc.tile_pool(name="psum", bufs=2, space="PSUM") as pspool:

                band_list = []
                for b in range(b_per):
                    for g in range(G):
                        for band in range(n_bands):
                            band_list.append((b, g, band))

                xtiles = {}
                PREFETCH = 5

                def emit_in_dma(i):
                    b, g, band = band_list[i]
                    r0 = band * rows_band
                    lo = max(r0 - 1, 0)
                    hi = min(r0 + rows_band + 1, h)
                    xt = xpool.tile([128, flat_n], F16)
                    t0 = lo - (r0 - 1)
                    if i == 0:
                        # WAW guards: force this DMA after the const loads so
                        # the tiny ident/wlt/zt transfers aren't starved by
                        # this 4MB transfer on the shared SDMA engines (the
                        # wd->diag chain gates all PE work).
                        nc.scalar.mul(xt[:, PAD + t0 * W: PAD + t0 * W + 1],
                                      ident[:, 0:1], 0.0)
                        nc.scalar.mul(xt[0:Z_DIM, PAD + t0 * W + 1:
                                         PAD + t0 * W + 2],
                                      wlt[:, 0:1], 0.0)
                        # chunked so the first groups' compute starts early
                        for clo, chi in ((lo, 17), (17, 41), (41, hi)):
                            tc0 = clo - (r0 - 1)
                            nc.gpsimd.dma_start(
                                xt[:, PAD + tc0 * W: PAD + (tc0 + (chi - clo)) * W],
                                x_d.ap()[b, 128 * g:128 * (g + 1), clo:chi, :])
                    else:
                        nc.gpsimd.dma_start(
                            xt[:, PAD + t0 * W: PAD + (t0 + (hi - lo)) * W],
                            x_d.ap()[b, 128 * g:128 * (g + 1), lo:hi, :])
                    xtiles[i] = xt
                    return xt

                for i, (b, g, band) in enumerate(band_list):
                    r0 = band * rows_band

                    if i == 0:
                        for j in range(PREFETCH):
                            emit_in_dma(j)
                    if i + PREFETCH < len(band_list):
                        emit_in_dma(i + PREFETCH)
                    xt = xtiles.pop(i)

                    # build this image's diag weights before its first band
                    if band == 0:
                        for ti in PE_TAPS:
                            col = (g * K * K + ti) * b_per + b
                            dtile = wpool.tile([128, 128], F16,
                                               tag=f"diag_{b}_{g}_{ti}")
                            nc.scalar.mul(dtile[:], ident[:], wd[:, col:col + 1])
                            diags[(b, g, ti)] = dtile

                    # zero pads: first/last image row, first-use flat pads
                    if i < 6:
                        nc.vector.memset(xt[:, 0:PAD], 0.0)
                        nc.vector.memset(
                            xt[:, PAD + tile_rows * W: PAD + tile_rows * W + 2], 0.0)
                    if r0 == 0:
                        nc.scalar.mul(xt[:, PAD:PAD + W], ident[:], 0.0)
                    if r0 + rows_band == h:
                        nc.scalar.mul(
                            xt[:, PAD + (tile_rows - 1) * W: PAD + tile_rows * W],
                            ident[:], 0.0)

                    ot = opool.tile([128, band_free], F16)

                    for grp in range(grp_per_band):
                        j0 = grp * rows_group

                        def win(ti, length):
                            dy, dx = TAPS[ti]
                            s = PAD + (j0 + dy) * W + dx - 1
                            return s, s + length

                        ps = pspool.tile([128, grp_free], F32)
                        rows_bank = 512 // W
                        for k, ti in enumerate(PE_TAPS):
                            dy, dx = TAPS[ti]
                            for bank in range(banks_per_grp):
                                s = PAD + (j0 + bank * rows_bank + dy) * W + dx - 1
                                nc.tensor.matmul(
                                    ps[:, 512 * bank:512 * (bank + 1)],
                                    diags[(b, g, ti)][:],
                                    xt[:, s:s + 512],
                                    start=(k == 0),
                                    stop=(k == len(PE_TAPS) - 1))

                        og = ot[:, j0 * W: j0 * W + grp_free]
                        # first write: DVE tensor_scalar (4x mode)
                        col = (g * K * K + TS_TAP) * b_per + b
                        s0, s1 = win(TS_TAP, grp_free)
                        nc.vector.tensor_scalar(
                            out=og, in0=xt[:, s0:s1],
                            scalar1=wd[:, col:col + 1], scalar2=None,
                            op0=AluOpType.mult)
                        # ACT taps into scratch, added by DVE tensor_tensor (2x)
                        for ti in ACT_TAPS:
                            col = (g * K * K + ti) * b_per + b
                            s0, s1 = win(ti, grp_free)
                            sc = scpool.tile([128, grp_free], F16)
                            nc.scalar.mul(sc[:], xt[:, s0:s1], wd[:, col:col + 1])
                            nc.vector.tensor_tensor(
                                out=og, in0=sc[:], in1=og, op=AluOpType.add)
                        # PSUM drain: half the groups via ACT copy + DVE 2x
                        # add (offloads the 1x psum-operand add from DVE and
                        # frees the PSUM buffer earlier); rest via DVE add.
                        if grp % 2 == 0:
                            sc3 = scpool.tile([128, grp_free], F16)
                            nc.scalar.copy(sc3[:], ps[:])
                            nc.vector.tensor_tensor(
                                out=og, in0=sc3[:], in1=og, op=AluOpType.add)
                        else:
                            nc.vector.tensor_tensor(
                                out=og, in0=ps[:], in1=og, op=AluOpType.add)

                    # width-edge wrap corrections:
                    # og[j, 0]   -= w[dy,0] * flat[PAD + (j+dy)*W - 1]
                    # og[j, W-1] -= w[dy,2] * flat[PAD + (j+dy+1)*W]
                    # (split per half-band on the final band to shrink the
                    # drain tail; whole-band otherwise)
                    otv = ot[:].rearrange("p (r c) -> p r c", c=W)
                    halves = ((0, 32), (32, 64)) if i == len(band_list) - 1 \
                        else ((0, 64),)
                    for h0, h1 in halves:
                        for dy in range(3):
                            for dx, off, oc in (
                                    (0, PAD + dy * W - 1, 0),
                                    (2, PAD + (dy + 1) * W, W - 1)):
                                ti = dy * 3 + dx
                                col = (g * K * K + ti) * b_per + b
                                in0 = (xt[:, off + h0 * W: off + h1 * W]
                                       .rearrange("p (r c) -> p r c", c=W)
                                       [:, :, 0:1])
                                oe = otv[:, h0:h1, oc:oc + 1]
                                nc.vector.scalar_tensor_tensor(
                                    out=oe, in0=in0,
                                    scalar=wdn[:, col:col + 1], in1=oe,
                                    op0=AluOpType.mult, op1=AluOpType.add)

                    # output DMAs (half-band each) with fp16->fp32 cast
                    for h0, h1 in ((0, 32), (32, 64)):
                        nc.gpsimd.dma_start(
                            y_d.ap()[b, 128 * g:128 * (g + 1),
                                     r0 + h0:r0 + h1, :],
                            ot[:, h0 * W:h1 * W])

    nc.compile()
    return nc


def make_in_maps(x, z, W_lin, b_per=B_PER):
    """Host-side shard + layout transforms (no math)."""
    wl = np.asarray(W_lin, dtype=np.float32)
    wlperm = (wl.reshape(G, 128, K * K, Z_DIM)
                .transpose(0, 2, 1, 3)
                .reshape(OUT_C * K * K, Z_DIM))
    wlt = np.ascontiguousarray(wlperm.T)                  # [64, 2304]
    ident = np.eye(128, dtype=np.float32)
    x = np.asarray(x, dtype=np.float32)
    z = np.asarray(z, dtype=np.float32)
    in_maps = []
    for c in range(N_CORES):
        sl = slice(c * b_per, (c + 1) * b_per)
        in_maps.append({
            "x": np.ascontiguousarray(x[sl]),
            "zT": np.ascontiguousarray(z[sl].T),          # [64, b_per]
            "wlt": wlt,
            "ident": ident,
        })
    return in_maps


_NC_CACHE = {}


def kernel(x, z, W_lin):
    key = "main"
    if key not in _NC_CACHE:
        _NC_CACHE[key] = build_nc()
    nc = _NC_CACHE[key]
    in_maps = make_in_maps(x, z, W_lin)
    res = bass_utils.run_bass_kernel_spmd(nc, in_maps, core_ids=list(range(N_CORES)))
    out = np.concatenate([res.results[c]["y"] for c in range(N_CORES)], axis=0)
    return out.astype(np.float32, copy=False)


# revision 9
# speedup vs baseline: 1.1023x; 1.1023x over previous
"""Trainium2 Bass kernel: per-sample hypernetwork depthwise 3x3 conv.

Reference computation (per batch b):
    W_dw[b] = (z[b] @ W_lin.T).reshape(OUT_C, 1, 3, 3)
    y[b]    = depthwise_conv2d(x[b], W_dw[b], padding=1)

Sharding: data-parallel over batch across 8 NeuronCores (2 batches/core),
W_lin replicated. Each core computes its own W_dw on-device.

Per-core v2 design (fp16 on-chip, fp32 in HBM):
  - channels (256) -> 2 groups of 128 on SBUF partitions
  - image split into 64-row bands; each band (66 rows incl 1-row halo each
    side) is DMA'd HBM->SBUF with an inline fp32->fp16 cast (SWDGE/gpsimd),
    into a flat tile (rows back to back, 2 leading pad elems). Width-edge
    taps wrap into the neighboring row; those wrong contributions are
    subtracted afterwards by small correction ops with negated weights.
  - per 16-row output group (2048 elems), the 9 conv taps split:
      * 6 taps on PE: diagonal-weight fp16 matmuls accumulating in PSUM
      * 1 tap on DVE tensor_scalar (4x perf mode) - first write of og
      * 2 taps on ACT (scalar engine) multiplies into scratch, added to og
        by DVE tensor_tensor (2x perf mode)
      * PSUM drained by one DVE tensor_tensor add (1x, psum operand)
  - output band DMA'd SBUF->HBM with inline fp16->fp32 cast (SWDGE)
  - W_dw computed on-device by 18 small fp32 matmuls from a host-side
    re-layout of W_lin (pure permutation/transpose, no host math)
"""

import os
import sys

for _p in ("/opt/trn_rl_repo", "/root/.axon_site", "/root/.axon_site/_ro/trn_rl_repo",
           "/root/.axon_site/_ro/pypackages"):
    if os.path.isdir(_p) and _p not in sys.path:
        sys.path.append(_p)

import numpy as np

import concourse.bass as bass
import concourse.tile as tile
from concourse import bacc, mybir
from concourse import bass_utils
from concourse.alu_op_type import AluOpType

F32 = mybir.dt.float32
F16 = mybir.dt.float16

# problem constants (hardcoded per contract)
B, OUT_C, H, W = 16, 256, 128, 128
K, Z_DIM = 3, 64
N_CORES = 8
B_PER = B // N_CORES          # 2 batches per core
G = OUT_C // 128              # 2 channel groups of 128

ROWS_BAND = 64
ROWS_GROUP = 16
PAD = 2                       # leading pad elems in the flat band tile

TAPS = [(dy, dx) for dy in range(3) for dx in range(3)]
PE_TAPS = [0, 2, 3, 5, 6, 8]  # dx in {0,2}
TS_TAP = 1                    # (dy=0, dx=1) -> DVE tensor_scalar first write
ACT_TAPS = [4, 7]             # (dy,dx)=(1,1),(2,1) -> ACT mul + DVE add


def build_nc(rows_band=ROWS_BAND, rows_group=ROWS_GROUP, b_per=B_PER, h=H):
    n_bands = h // rows_band
    grp_per_band = rows_band // rows_group
    banks_per_grp = rows_group * W // 512   # 4
    grp_free = rows_group * W               # 2048
    band_free = rows_band * W               # 8192
    tile_rows = rows_band + 2               # 66
    flat_n = PAD + tile_rows * W + 2 + W    # slack for correction APs

    nc = bacc.Bacc("TRN2", target_bir_lowering=False, debug=False)

    x_d = nc.dram_tensor("x", [b_per, OUT_C, h, W], F32, kind="ExternalInput")
    zt_d = nc.dram_tensor("zT", [Z_DIM, b_per], F32, kind="ExternalInput")
    wlt_d = nc.dram_tensor("wlt", [Z_DIM, OUT_C * K * K], F32, kind="ExternalInput")
    ident_d = nc.dram_tensor("ident", [128, 128], F32, kind="ExternalInput")
    y_d = nc.dram_tensor("y", [b_per, OUT_C, h, W], F32, kind="ExternalOutput")

    n_chunks = OUT_C * K * K // 128          # 18
    wd_cols = K * K * G * b_per              # 36, col = (g*9 + t)*b_per + b

    with tile.TileContext(nc) as tc:
        with tc.tile_pool(name="wconst", bufs=1) as wpool:
            ident = wpool.tile([128, 128], F32)
            nc.sync.dma_start(ident[:], ident_d.ap()[:, :])
            wlt = wpool.tile([Z_DIM, OUT_C * K * K], F32)
            half = OUT_C * K * K // 2
            nc.sync.dma_start(wlt[:, 0:half], wlt_d.ap()[:, 0:half])
            nc.sync.dma_start(wlt[:, half:], wlt_d.ap()[:, half:])
            zt = wpool.tile([Z_DIM, b_per], F32)
            nc.sync.dma_start(zt[:], zt_d.ap()[:, :])

            wd = wpool.tile([128, wd_cols], F32)
            with tc.tile_pool(name="wpsum", bufs=2, space="PSUM") as wps:
                for j in range(n_chunks):
                    ps = wps.tile([128, b_per], F32)
                    nc.tensor.matmul(ps[:], wlt[:, 128 * j:128 * (j + 1)], zt[:],
                                     start=True, stop=True)
                    nc.scalar.copy(wd[:, b_per * j:b_per * (j + 1)], ps[:])

            # negated weights for the wrap corrections
            wdn = wpool.tile([128, wd_cols], F32)
            nc.scalar.mul(wdn[:], wd[:], -1.0)

            # fp16 diagonal weight matrices for the PE taps (built lazily
            # per (b,g) right before that image's bands, see loop below)
            diags = {}

            with tc.tile_pool(name="xband", bufs=5) as xpool, \
                 tc.tile_pool(name="oband", bufs=4) as opool, \
                 tc.tile_pool(name="scr", bufs=5) as scpool, \
                 tc.tile_pool(name="psum", bufs=2, space="PSUM") as pspool:

                band_list = []
                for b in range(b_per):
                    for g in range(G):
                        for band in range(n_bands):
                            band_list.append((b, g, band))

                xtiles = {}
                PREFETCH = 3

                def emit_in_dma(i):
                    b, g, band = band_list[i]
                    r0 = band * rows_band
                    lo = max(r0 - 1, 0)
                    hi = min(r0 + rows_band + 1, h)
                    xt = xpool.tile([128, flat_n], F16)
                    t0 = lo - (r0 - 1)
                    if i == 0:
                        # WAW guard: force this DMA after zt - the LAST const
                        # on the sync HWDGE ring (FIFO), so its completion
                        # implies ident/wlt/zt all drained. Without it this
                        # 4MB transfer starves the tiny const loads on the
                        # shared SDMA engines (the wd->diag chain gates all
                        # PE work).
                        nc.scalar.mul(xt[0:Z_DIM, PAD + t0 * W:
                                         PAD + t0 * W + 1],
                                      zt[:, 0:1], 0.0)
                        # chunked so the first groups' compute starts early
                        for clo, chi in ((lo, 17), (17, 41), (41, hi)):
                            tc0 = clo - (r0 - 1)
                            nc.gpsimd.dma_start(
                                xt[:, PAD + tc0 * W: PAD + (tc0 + (chi - clo)) * W],
                                x_d.ap()[b, 128 * g:128 * (g + 1), clo:chi, :])
                    else:
                        nc.gpsimd.dma_start(
                            xt[:, PAD + t0 * W: PAD + (t0 + (hi - lo)) * W],
                            x_d.ap()[b, 128 * g:128 * (g + 1), lo:hi, :])
                    xtiles[i] = xt
                    return xt

                for i, (b, g, band) in enumerate(band_list):
                    r0 = band * rows_band

                    if i == 0:
                        for j in range(PREFETCH):
                            emit_in_dma(j)
                    if i + PREFETCH < len(band_list):
                        emit_in_dma(i + PREFETCH)
                    xt = xtiles.pop(i)

                    # build this image's diag weights before its first band
                    if band == 0:
                        for ti in PE_TAPS:
                            col = (g * K * K + ti) * b_per + b
                            dtile = wpool.tile([128, 128], F16,
                                               tag=f"diag_{b}_{g}_{ti}")
                            nc.scalar.mul(dtile[:], ident[:], wd[:, col:col + 1])
                            diags[(b, g, ti)] = dtile

                    # zero pads: first/last image row, first-use flat pads
                    if i < 6:
                        nc.vector.memset(xt[:, 0:PAD], 0.0)
                        nc.vector.memset(
                            xt[:, PAD + tile_rows * W: PAD + tile_rows * W + 2], 0.0)
                    if r0 == 0:
                        nc.scalar.mul(xt[:, PAD:PAD + W], ident[:], 0.0)
                    if r0 + rows_band == h:
                        nc.scalar.mul(
                            xt[:, PAD + (tile_rows - 1) * W: PAD + tile_rows * W],
                            ident[:], 0.0)

                    ot = opool.tile([128, band_free], F16)

                    for grp in range(grp_per_band):
                        j0 = grp * rows_group

                        def win(ti, length):
                            dy, dx = TAPS[ti]
                            s = PAD + (j0 + dy) * W + dx - 1
                            return s, s + length

                        ps = pspool.tile([128, grp_free], F32)
                        rows_bank = 512 // W
                        for k, ti in enumerate(PE_TAPS):
                            dy, dx = TAPS[ti]
                            for bank in range(banks_per_grp):
                                s = PAD + (j0 + bank * rows_bank + dy) * W + dx - 1
                                nc.tensor.matmul(
                                    ps[:, 512 * bank:512 * (bank + 1)],
                                    diags[(b, g, ti)][:],
                                    xt[:, s:s + 512],
                                    start=(k == 0),
                                    stop=(k == len(PE_TAPS) - 1))

                        og = ot[:, j0 * W: j0 * W + grp_free]
                        # first write: DVE tensor_scalar (4x mode)
                        col = (g * K * K + TS_TAP) * b_per + b
                        s0, s1 = win(TS_TAP, grp_free)
                        nc.vector.tensor_scalar(
                            out=og, in0=xt[:, s0:s1],
                            scalar1=wd[:, col:col + 1], scalar2=None,
                            op0=AluOpType.mult)
                        # ACT taps into scratch, added by DVE tensor_tensor (2x)
                        for ti in ACT_TAPS:
                            col = (g * K * K + ti) * b_per + b
                            s0, s1 = win(ti, grp_free)
                            sc = scpool.tile([128, grp_free], F16)
                            nc.scalar.mul(sc[:], xt[:, s0:s1], wd[:, col:col + 1])
                            nc.vector.tensor_tensor(
                                out=og, in0=sc[:], in1=og, op=AluOpType.add)
                        # PSUM drain: half the groups via ACT copy + DVE 2x
                        # add (offloads the 1x psum-operand add from DVE and
                        # frees the PSUM buffer earlier); rest via DVE add.
                        if grp % 2 == 0:
                            sc3 = scpool.tile([128, grp_free], F16)
                            nc.scalar.copy(sc3[:], ps[:])
                            nc.vector.tensor_tensor(
                                out=og, in0=sc3[:], in1=og, op=AluOpType.add)
                        else:
                            nc.vector.tensor_tensor(
                                out=og, in0=ps[:], in1=og, op=AluOpType.add)

                    # width-edge wrap corrections:
                    # og[j, 0]   -= w[dy,0] * flat[PAD + (j+dy)*W - 1]
                    # og[j, W-1] -= w[dy,2] * flat[PAD + (j+dy+1)*W]
                    # (split per half-band on the final band to shrink the
                    # drain tail; whole-band otherwise)
                    otv = ot[:].rearrange("p (r c) -> p r c", c=W)
                    halves = ((0, 32), (32, 64)) if i == len(band_list) - 1 \
                        else ((0, 64),)
                    for h0, h1 in halves:
                        for dy in range(3):
                            for dx, off, oc in (
                                    (0, PAD + dy * W - 1, 0),
                                    (2, PAD + (dy + 1) * W, W - 1)):
                                ti = dy * 3 + dx
                                col = (g * K * K + ti) * b_per + b
                                in0 = (xt[:, off + h0 * W: off + h1 * W]
                                       .rearrange("p (r c) -> p r c", c=W)
                                       [:, :, 0:1])
                                oe = otv[:, h0:h1, oc:oc + 1]
                                nc.vector.scalar_tensor_tensor(
                                    out=oe, in0=in0,
                                    scalar=wdn[:, col:col + 1], in1=oe,
                                    op0=AluOpType.mult, op1=AluOpType.add)

                    # output DMAs (half-band each) with fp16->fp32 cast
                    for h0, h1 in ((0, 32), (32, 64)):
                        nc.gpsimd.dma_start(
                            y_d.ap()[b, 128 * g:128 * (g + 1),
                                     r0 + h0:r0 + h1, :],
                            ot[:, h0 * W:h1 * W])

    nc.compile()
    return nc


def make_in_maps(x, z, W_lin, b_per=B_PER):
    """Host-side shard + layout transforms (no math)."""
    wl = np.asarray(W_lin, dtype=np.float32)
    wlperm = (wl.reshape(G, 128, K * K, Z_DIM)
                .transpose(0, 2, 1, 3)
                .reshape(OUT_C * K * K, Z_DIM))
    wlt = np.ascontiguousarray(wlperm.T)                  # [64, 2304]
    ident = np.eye(128, dtype=np.float32)
    x = np.asarray(x, dtype=np.float32)
    z = np.asarray(z, dtype=np.float32)
    in_maps = []
    for c in range(N_CORES):
        sl = slice(c * b_per, (c + 1) * b_per)
        in_maps.append({
            "x": np.ascontiguousarray(x[sl]),
            "zT": np.ascontiguousarray(z[sl].T),          # [64, b_per]
            "wlt": wlt,
            "ident": ident,
        })
    return in_maps


_NC_CACHE = {}


def kernel(x, z, W_lin):
    key = "main"
    if key not in _NC_CACHE:
        _NC_CACHE[key] = build_nc()
    nc = _NC_CACHE[key]
    in_maps = make_in_maps(x, z, W_lin)
    res = bass_utils.run_bass_kernel_spmd(nc, in_maps, core_ids=list(range(N_CORES)))
    out = np.concatenate([res.results[c]["y"] for c in range(N_CORES)], axis=0)
    return out.astype(np.float32, copy=False)
